# revision 2
# baseline (speedup 1.0000x reference)
"""Trainium2 Bass kernel v2 for nn_LoraLinear (8 cores, data-parallel).

Same math as the baseline, but the per-row LoRA-B table rows are
pre-gathered on the host in batch order, so the device does straight
streaming DMA loads instead of indirect gathers from a replicated
10000-row table. Upload per core drops from ~654 MB to ~35 MB (bf16)
or ~19 MB (fp8 tables with a x64 scale folded into the `a` operand).

Per batch row b:
    out[b] = x[b] @ W_base.T + b_base
             + S * ( (B_user[u_b] + B_item[i_b] + W_common) @ (x[b] @ A.T) )
"""
import numpy as np
import ml_dtypes

import concourse.bass as bass
import concourse.bacc as bacc
import concourse.tile as tile
from concourse import mybir
from concourse.bass_utils import run_bass_kernel_spmd

IN_F = 1024
OUT_F = 1024
R = 16
BATCH = 4096
SCALING = 2.0
N_CORES = 8

B_SH = BATCH // N_CORES          # 512 rows per core
RG = 64                          # batch rows packed per matmul group
S_SUB = 2                        # r-halves per partition dim (RG * S_SUB = 128)
C_SUB = R // S_SUB               # 8 r-chunks accumulated via separate matmuls
NG = B_SH // RG                  # 8 groups per core
NKC = IN_F // 128                # 8 contraction chunks for the base matmul
NH = OUT_F // 512                # 2 output halves (PSUM bank free-dim limit)
NBG = B_SH // 128                # 4 PSUM row-blocks

F32 = mybir.dt.float32
BF16 = mybir.dt.bfloat16
FP8 = mybir.dt.float8e3
I32 = mybir.dt.int32

TAB_SCALE = 64.0                 # fp8 variant: tables x64, a2 /64

_CACHE = {}


def _build(tab_dt=FP8):
    nc = bacc.Bacc("TRN2", target_bir_lowering=False, debug=False,
                   num_devices=N_CORES)
    xt = nc.dram_tensor("xt", [IN_F, B_SH], BF16, kind="ExternalInput")
    wt = nc.dram_tensor("wt", [IN_F, OUT_F], BF16, kind="ExternalInput")
    a2w = nc.dram_tensor("a2w", [128, NKC * R], BF16, kind="ExternalInput")
    wct = nc.dram_tensor("wct", [R, OUT_F], F32, kind="ExternalInput")
    biasb = nc.dram_tensor("biasb", [1, OUT_F], BF16, kind="ExternalInput")
    ones1 = nc.dram_tensor("ones1", [1, 128], BF16, kind="ExternalInput")
    ltab = nc.dram_tensor("ltab", [R, C_SUB * 128], F32, kind="ExternalInput")
    masks = nc.dram_tensor("masks", [128, RG], F32, kind="ExternalInput")
    # pre-gathered tables, batch order: macro-row 2*j+s holds r-half s of
    # batch row j's B row, as C_SUB chunks of OUT_F values
    but = nc.dram_tensor("but", [B_SH * S_SUB, C_SUB * OUT_F], tab_dt,
                         kind="ExternalInput")
    bit = nc.dram_tensor("bit", [B_SH * S_SUB, C_SUB * OUT_F], tab_dt,
                         kind="ExternalInput")
    y = nc.dram_tensor("y", [B_SH, OUT_F], BF16, kind="ExternalOutput")

    with tile.TileContext(nc) as tc:
        with (
            tc.tile_pool(name="const", bufs=1) as cp,
            tc.tile_pool(name="gath", bufs=4) as gp,
            tc.tile_pool(name="btp", bufs=16) as btp,
            tc.tile_pool(name="ps", bufs=8, space="PSUM") as psp,
            tc.tile_pool(name="outp", bufs=3) as op,
        ):
            # ---- constant / weight loads (once) ----
            xt_t = []
            for k in range(NKC):
                t = cp.tile([128, B_SH], BF16, tag=f"xt{k}")
                nc.sync.dma_start(t[:], xt.ap()[128 * k:128 * (k + 1), :])
                xt_t.append(t)
            wt_t = []
            for k in range(NKC):
                t = cp.tile([128, OUT_F], BF16, tag=f"wt{k}")
                nc.sync.dma_start(t[:], wt.ap()[128 * k:128 * (k + 1), :])
                wt_t.append(t)
            a2w_t = cp.tile([128, NKC * R], BF16, tag="a2w")
            nc.sync.dma_start(a2w_t[:], a2w.ap())
            wct_t = cp.tile([R, OUT_F], F32, tag="wct")
            nc.sync.dma_start(wct_t[:], wct.ap())
            bias_t = cp.tile([1, OUT_F], BF16, tag="bias")
            nc.sync.dma_start(bias_t[:], biasb.ap())
            ones_t = cp.tile([1, 128], BF16, tag="ones")
            nc.sync.dma_start(ones_t[:], ones1.ap())
            ltab_t = cp.tile([R, C_SUB * 128], F32, tag="ltab")
            nc.sync.dma_start(ltab_t[:], ltab.ap())
            mask_t = cp.tile([128, RG], F32, tag="mask")
            nc.sync.dma_start(mask_t[:], masks.ap())

            # ---- a2T = (2A) @ x_shard.T  -> [16, 512] f32 ----
            a2t_ps = psp.tile([R, B_SH], F32, tag="ps", space="PSUM")
            for k in range(NKC):
                nc.tensor.matmul(
                    a2t_ps[:], lhsT=a2w_t[:, R * k:R * (k + 1)],
                    rhs=xt_t[k][:],
                    start=(k == 0), stop=(k == NKC - 1),
                    skip_group_check=True)
            a2t_sb = cp.tile([R, B_SH], F32, tag="a2t")
            nc.vector.tensor_copy(a2t_sb[:], a2t_ps[:])

            # ---- rep_c: a2 values repacked for the block-diag lhsT ----
            rep_sb = []
            for c in range(C_SUB):
                rps = psp.tile([128, B_SH], F32, tag="ps", space="PSUM")
                nc.tensor.matmul(
                    rps[:], lhsT=ltab_t[:, 128 * c:128 * (c + 1)],
                    rhs=a2t_sb[:],
                    start=True, stop=True, skip_group_check=True)
                rsb = cp.tile([128, B_SH], F32, tag=f"rep{c}")
                nc.vector.tensor_copy(rsb[:], rps[:])
                rep_sb.append(rsb)

            # ---- output PSUM banks: bias + base + common ----
            out_ps = {}
            for bg in range(NBG):
                for h in range(NH):
                    ps = psp.tile([128, 512], F32, tag="ps", space="PSUM")
                    out_ps[(bg, h)] = ps
                    nc.tensor.matmul(  # bias broadcast (K=1)
                        ps[:], lhsT=ones_t[:],
                        rhs=bias_t[:, 512 * h:512 * h + 512],
                        start=True, stop=False, skip_group_check=True)
                    for k in range(NKC):  # base: x @ W_base.T (bf16)
                        nc.tensor.matmul(
                            ps[:], lhsT=xt_t[k][:, 128 * bg:128 * (bg + 1)],
                            rhs=wt_t[k][:, 512 * h:512 * h + 512],
                            start=False, stop=False, skip_group_check=True)
                    nc.tensor.matmul(  # common: a2 @ W_common.T
                        ps[:], lhsT=a2t_sb[:, 128 * bg:128 * (bg + 1)],
                        rhs=wct_t[:, 512 * h:512 * h + 512],
                        start=False, stop=False, skip_group_check=True)

            # ---- streaming table loads + block-diagonal matmuls ----
            n_left = {k: 2 * NH * C_SUB for k in out_ps}
            gpb = 128 // RG          # groups per 128-row PSUM bank
            for g in range(NG):
                bg, strip = g // gpb, (g % gpb) * RG
                bts = []
                for c in range(C_SUB):
                    bt = btp.tile([128, RG], BF16, tag="bt")
                    nc.vector.tensor_tensor(
                        out=bt[:], in0=mask_t[:],
                        in1=rep_sb[c][:, RG * g:RG * (g + 1)],
                        op=mybir.AluOpType.mult)
                    bts.append(bt)
                for tab in (but, bit):
                    gt = gp.tile([128, C_SUB * OUT_F], tab_dt, tag="gt")
                    nc.sync.dma_start(
                        gt[:], tab.ap()[128 * g:128 * (g + 1), :])
                    for c in range(C_SUB):
                        for h in range(NH):
                            key = (bg, h)
                            n_left[key] -= 1
                            nc.tensor.matmul(
                                out_ps[key][strip:strip + RG, :],
                                lhsT=bts[c][:],
                                rhs=gt[:, OUT_F * c + 512 * h:
                                       OUT_F * c + 512 * h + 512],
                                start=False, stop=(n_left[key] == 0),
                                tile_position=(0, strip),
                                skip_group_check=True)

            # ---- PSUM -> SBUF -> DRAM ----
            for bg in range(NBG):
                for h in range(NH):
                    ot = op.tile([128, 512], BF16, tag="ot")
                    nc.scalar.copy(ot[:], out_ps[(bg, h)][:])
                    nc.sync.dma_start(
                        y.ap()[128 * bg:128 * (bg + 1),
                               512 * h:512 * h + 512],
                        ot[:])
    nc.compile()
    return nc


def _prep_host(x, user_indices, item_indices, W_base, b_base, A, B_user,
               B_item, W_common, tab_dt_np=None, tab_scale=TAB_SCALE):
    """Host-side layout prep. Returns (shared dict, per-core list of dicts)."""
    bf16 = ml_dtypes.bfloat16
    if tab_dt_np is None:
        tab_dt_np = ml_dtypes.float8_e3m4
    x = np.asarray(x, np.float32)
    W_base = np.asarray(W_base, np.float32)
    b_base = np.asarray(b_base, np.float32)
    A = np.asarray(A, np.float32)
    W_common = np.asarray(W_common, np.float32)
    user_indices = np.asarray(user_indices, np.int32)
    item_indices = np.asarray(item_indices, np.int32)

    wt = np.ascontiguousarray(W_base.T).astype(bf16)          # [in, out]
    a2t = np.ascontiguousarray((SCALING / tab_scale * A).T)   # [in, R]
    a2w = np.ascontiguousarray(
        a2t.reshape(NKC, 128, R).transpose(1, 0, 2).reshape(128, NKC * R)
    ).astype(bf16)
    # a2 carries SCALING/tab_scale, so the common projection needs x tab_scale
    wct = np.ascontiguousarray((tab_scale * W_common).T)  # [R, out] f32
    biasb = b_base.reshape(1, OUT_F).astype(bf16)
    ones1 = np.ones((1, 128), bf16)
    # ltab[r, 128c + p] = 1 if r == C_SUB*(p % S_SUB) + c
    ltab = np.zeros((R, C_SUB * 128), np.float32)
    p = np.arange(128)
    for c in range(C_SUB):
        ltab[C_SUB * (p % S_SUB) + c, 128 * c + p] = 1.0
    # masks[p, j] = 1 if p // S_SUB == j
    masks = np.zeros((128, RG), np.float32)
    masks[p, p // S_SUB] = 1.0

    # pre-gather table rows in batch order:
    # [4096,out,R] -> [4096,R,out] -> macro rows [4096*S, C*out]
    gat_u = np.ascontiguousarray(
        (np.asarray(B_user, np.float32)[user_indices] * tab_scale)
        .transpose(0, 2, 1)).astype(tab_dt_np).reshape(BATCH * S_SUB,
                                                       C_SUB * OUT_F)
    gat_i = np.ascontiguousarray(
        (np.asarray(B_item, np.float32)[item_indices] * tab_scale)
        .transpose(0, 2, 1)).astype(tab_dt_np).reshape(BATCH * S_SUB,
                                                       C_SUB * OUT_F)

    shared = dict(wt=wt, a2w=a2w, wct=wct, biasb=np.asarray(biasb),
                  ones1=np.asarray(ones1), ltab=ltab, masks=masks)
    per_core = []
    for c in range(N_CORES):
        sl = slice(B_SH * c, B_SH * (c + 1))
        msl = slice(B_SH * S_SUB * c, B_SH * S_SUB * (c + 1))
        xt_c = np.ascontiguousarray(x[sl].T).astype(bf16)     # [in, 512]
        per_core.append(dict(xt=xt_c, but=gat_u[msl], bit=gat_i[msl]))
    return shared, per_core


def kernel(**inputs) -> np.ndarray:
    if "nc" not in _CACHE:
        _CACHE["nc"] = _build()
    nc = _CACHE["nc"]
    shared, per_core = _prep_host(**inputs)
    in_maps = [{**shared, **pc} for pc in per_core]
    res = run_bass_kernel_spmd(nc, in_maps, core_ids=list(range(N_CORES)))
    out = np.concatenate([res.results[c]["y"] for c in range(N_CORES)], axis=0)
    return out.astype(np.float32)
